# revision 1
# baseline (speedup 1.0000x reference)
"""Trainium2 Bass kernel for the Doppler channel problem.

Math (per batch row n, antenna p):
    weight[n,p,s] = sum_l cof[n,p,l] * shift[l,s]          (complex, L=16, S=14)
    out[n,p,s,k]  = x[n,p,s,k] * weight[n,p,s]             (broadcast over MK=80)
    H_t[n,p,m]    = sum_l cof[n,p,l] * F[l,m]              (64-pt DFT of zero-padded taps)

Sharding: pure data parallelism over the batch dim N (1024) across 8 cores;
each core handles 128 batch rows = 1024 (n,p) rows. No cross-core comms.

Device layout per core: (n,p) rows go to SBUF partitions, 2 rows per
partition (chunk of 256 rows -> one (128, 2240) tile, 4 chunks per core).
The tiny complex matmuls (weight, H_t) run on the tensor engine with the
stacked-[cof_r; cof_i] trick: one fp32 matmul per (chunk, row-parity) with a
host-provided constant rhs produces [w_r | -w_i | H_r | H_i] in PSUM.
The broadcast multiply runs on the vector engine as per-(row,s) 2x-mode
tensor_scalar products plus two full-width combines.
"""

import numpy as np

import concourse.bass as bass
import concourse.tile as tile
from concourse import bacc, mybir
from concourse.bass_utils import run_bass_kernel_spmd

# Problem constants (hardcoded per spec nn_Channel_86947317940845)
L = 16
M = 64
N_PILOT = 2
V = 100.0
N, P, SMK = 1024, 8, 1120
NC = 8          # cores
RPC = N // NC   # batch rows per core (128)
ROWS = RPC * P  # (n,p) rows per core (1024)
G = 2           # (n,p) rows per partition
CHUNK = 128 * G      # rows per chunk (256)
NCHUNK = ROWS // CHUNK  # 4
FD = G * SMK         # free dim per tile (2240)

_F32 = mybir.dt.float32


def _constants(S, MK):
    """Host-side constant matrices: rhs for the PE matmul.

    rhs_all (2L, 2S + 2M) fp32, for stacked lhsT = [cof_r; cof_i]:
      cols [0:S)        -> w_r    = cof_r@s_r - cof_i@s_i   : [s_r; -s_i]
      cols [S:2S)       -> -w_i   = -(cof_r@s_i + cof_i@s_r): [-s_i; -s_r]
      cols [2S:2S+M)    -> H_r    = cof_r@F_r - cof_i@F_i   : [F_r; -F_i]
      cols [2S+M:2S+2M) -> H_i    = cof_r@F_i + cof_i@F_r   : [F_i; F_r]
    """
    t = np.linspace(0.0, (S - 1) * (0.0005 / 14.0), S)
    fd_max = V / 3.0e8 * 3.0e9
    angles = np.linspace(0.0, 2.0 * np.pi, L)
    phases = np.outer(2.0 * np.pi * np.cos(angles) * fd_max, t)  # (L,S)
    sr, si = np.cos(phases), np.sin(phases)
    lm = np.outer(np.arange(L), np.arange(M)) * (2.0 * np.pi / M)
    fr, fi = np.cos(lm), -np.sin(lm)  # F = exp(-2i pi l m / M)
    top = np.concatenate([sr, -si, fr, fi], axis=1)
    bot = np.concatenate([-si, -sr, -fi, fr], axis=1)
    return np.concatenate([top, bot], axis=0).astype(np.float32)  # (32, 2S+2M)


def _build(S, MK):
    RHS_W = 2 * S          # 28
    RHS_ALL = 2 * S + 2 * M  # 156
    HW = 2 * M             # H row width per (n,p) row (128)

    nc = bacc.Bacc("TRN2", target_bir_lowering=False, debug=False, num_devices=NC)
    xr = nc.dram_tensor("xr", [ROWS, SMK], _F32, kind="ExternalInput").ap()
    xi = nc.dram_tensor("xi", [ROWS, SMK], _F32, kind="ExternalInput").ap()
    ct = nc.dram_tensor("ct", [2 * L, ROWS], _F32, kind="ExternalInput").ap()
    rhs = nc.dram_tensor("rhs", [2 * L, RHS_ALL], _F32, kind="ExternalInput").ap()
    our = nc.dram_tensor("our", [ROWS, SMK], _F32, kind="ExternalOutput").ap()
    oui = nc.dram_tensor("oui", [ROWS, SMK], _F32, kind="ExternalOutput").ap()
    ht = nc.dram_tensor("ht", [ROWS, HW], _F32, kind="ExternalOutput").ap()

    # chunk views: partition p of chunk i holds rows i*CHUNK + G*p + g
    xr_v = xr.rearrange("(i p a) m -> i p (a m)", p=128, a=G)
    xi_v = xi.rearrange("(i p a) m -> i p (a m)", p=128, a=G)
    our_v = our.rearrange("(i p a) m -> i p (a m)", p=128, a=G)
    oui_v = oui.rearrange("(i p a) m -> i p (a m)", p=128, a=G)
    ht_v = ht.rearrange("(i p a) m -> i p (a m)", p=128, a=G)

    with tile.TileContext(nc) as tc:
        with (
            tc.tile_pool(name="consts", bufs=1) as consts,
            tc.tile_pool(name="xs", bufs=3) as xs,
            tc.tile_pool(name="ts", bufs=2) as tpool,
            tc.tile_pool(name="os", bufs=2) as opool,
            tc.tile_pool(name="ws", bufs=2) as wpool,
            tc.tile_pool(name="hs", bufs=2) as hpool,
            tc.tile_pool(name="psum", bufs=4, space="PSUM") as psum,
        ):
            ct_sb = consts.tile([2 * L, ROWS], _F32)
            nc.sync.dma_start(out=ct_sb[:], in_=ct[:])
            rhs_sb = consts.tile([2 * L, RHS_ALL], _F32)
            nc.sync.dma_start(out=rhs_sb[:], in_=rhs[:])

            for i in range(NCHUNK):
                xr_t = xs.tile([128, FD], _F32, tag="xr")
                nc.sync.dma_start(out=xr_t[:], in_=xr_v[i])
                xi_t = xs.tile([128, FD], _F32, tag="xi")
                nc.sync.dma_start(out=xi_t[:], in_=xi_v[i])

                w_t = wpool.tile([128, G * RHS_W], _F32)
                h_t = hpool.tile([128, G * HW], _F32)
                for g in range(G):
                    pw = psum.tile([128, RHS_ALL], _F32, tag="pw")
                    nc.tensor.matmul(
                        pw[:],
                        ct_sb[:, (G * i + g) * 128 : (G * i + g + 1) * 128],
                        rhs_sb[:],
                        start=True,
                        stop=True,
                    )
                    nc.vector.tensor_copy(
                        w_t[:, g * RHS_W : (g + 1) * RHS_W], pw[:, 0:RHS_W]
                    )
                    nc.scalar.copy(
                        h_t[:, g * HW : (g + 1) * HW], pw[:, RHS_W : RHS_W + HW]
                    )
                nc.sync.dma_start(out=ht_v[i], in_=h_t[:])

                t1 = tpool.tile([128, FD], _F32, tag="t1")  # -xi*wi
                t2 = tpool.tile([128, FD], _F32, tag="t2")  # xr*wr
                t3 = tpool.tile([128, FD], _F32, tag="t3")  # -xr*wi
                t4 = tpool.tile([128, FD], _F32, tag="t4")  # xi*wr
                for g in range(G):
                    for s in range(S):
                        sl = slice(g * SMK + s * MK, g * SMK + (s + 1) * MK)
                        wr = w_t[:, g * RHS_W + s : g * RHS_W + s + 1]
                        win = w_t[:, g * RHS_W + S + s : g * RHS_W + S + s + 1]
                        nc.vector.tensor_scalar_mul(t1[:, sl], xi_t[:, sl], win)
                        nc.vector.tensor_scalar_mul(t2[:, sl], xr_t[:, sl], wr)
                        nc.vector.tensor_scalar_mul(t3[:, sl], xr_t[:, sl], win)
                        nc.vector.tensor_scalar_mul(t4[:, sl], xi_t[:, sl], wr)

                our_t = opool.tile([128, FD], _F32, tag="our")
                nc.vector.tensor_add(our_t[:], t2[:], t1[:])
                oui_t = opool.tile([128, FD], _F32, tag="oui")
                nc.vector.tensor_sub(oui_t[:], t4[:], t3[:])
                nc.sync.dma_start(out=our_v[i], in_=our_t[:])
                nc.sync.dma_start(out=oui_v[i], in_=oui_t[:])

    nc.compile()
    return nc


_CACHE = {}


def _get_nc(S, MK):
    key = (S, MK)
    if key not in _CACHE:
        _CACHE[key] = _build(S, MK)
    return _CACHE[key]


def _in_maps(input_real, input_imag, cof_real, cof_imag, S, MK):
    rhs = _constants(S, MK)
    maps = []
    for c in range(NC):
        sl = slice(c * RPC, (c + 1) * RPC)
        xr = np.ascontiguousarray(input_real[sl]).reshape(ROWS, SMK)
        xi = np.ascontiguousarray(input_imag[sl]).reshape(ROWS, SMK)
        cr = np.ascontiguousarray(cof_real[sl]).reshape(ROWS, L)
        ci = np.ascontiguousarray(cof_imag[sl]).reshape(ROWS, L)
        cs = np.concatenate([cr.T, ci.T], axis=0)  # (2L, ROWS)
        # permute columns so block (G*i+g) holds rows i*CHUNK + G*p + g
        cs = (
            cs.reshape(2 * L, NCHUNK, 128, G)
            .transpose(0, 1, 3, 2)
            .reshape(2 * L, ROWS)
        )
        cs = np.ascontiguousarray(cs)
        maps.append({"xr": xr, "xi": xi, "ct": cs, "rhs": rhs})
    return maps


def kernel(input_real, input_imag, cof_real, cof_imag, Ns):
    S = int(Ns) + N_PILOT
    MK = SMK // S
    assert S * MK == SMK and S == 14

    nc = _get_nc(S, MK)
    maps = _in_maps(input_real, input_imag, cof_real, cof_imag, S, MK)
    res = run_bass_kernel_spmd(nc, maps, core_ids=list(range(NC)))

    out = np.empty((N, P, SMK), dtype=np.complex64)
    H_t = np.empty((N, P, M), dtype=np.complex64)
    for c in range(NC):
        r = res.results[c]
        sl = slice(c * RPC, (c + 1) * RPC)
        out[sl] = (r["our"] + 1j * r["oui"]).reshape(RPC, P, SMK)
        hh = r["ht"].reshape(RPC, P, 2 * M)
        H_t[sl] = hh[..., :M] + 1j * hh[..., M:]
    return out, H_t
